# revision 14
# baseline (speedup 1.0000x reference)
"""ACT (Adaptive Computation Time) transformer layer on 8 TRN2 NeuronCores.

Data-parallel over batch B=8 (one batch row per core). Each core runs the
full 4-step ACT recurrence on its 2048 tokens; the ponder mean is assembled
host-side from per-token ponder rows (no collectives needed).

Per-core layout: h lives TRANSPOSED (D on partitions, tokens on the free
dim) for the whole kernel. The host pre-transposes each core's x shard to
(D, T) and un-transposes the (D, T) output, so the kernel needs no on-chip
transposes at all. Each step's matmul is h_new^T[e,t] = sum_d W[d,e] h^T[d,t]
with W 128x128 blocks as the stationary operand (float32r, full PE rate),
accumulating over d in PSUM; gelu(+bias) is fused into the PSUM->SBUF
eviction on the scalar engine. The halting matvec (D->1) runs on the PE
with M=1 stationary columns, sigmoid fused on its eviction. The per-token
halting state machine runs on (1, T) rows on the DVE; the per-token weight
is broadcast across partitions (gpsimd.partition_broadcast) for the
out^T += w * h^T accumulation. Tokens are processed in 2 panels of 1024 to
fit SBUF; weights stream per (panel, step), double-buffered in halves.
"""

import os
import sys

import numpy as np

if "/opt/trn_rl_repo" not in sys.path:
    sys.path.insert(0, "/opt/trn_rl_repo")

os.environ.setdefault("MYCRO_LOCAL_CACHE", "1")

B, T, D, L = 8, 2048, 1024, 4
THRESHOLD = 0.99
N_CORES = 8

R = 4            # token panels per core
TR = T // R      # tokens per panel
ND = D // 128    # 128-partition tiles along D
CH = 512         # matmul moving free-dim chunk (fp32 PSUM bank)
NCH = TR // CH   # chunks per panel

_CACHED = None


def _build():
    import concourse.bacc as bacc
    import concourse.mybir as mybir
    import concourse.tile as tile

    F32 = mybir.dt.float32
    F32R = mybir.dt.float32r
    I32 = mybir.dt.int32
    AF = mybir.ActivationFunctionType
    ALU = mybir.AluOpType

    nc = bacc.Bacc("TRN2", target_bir_lowering=False, debug=False,
                   num_devices=N_CORES)

    # x arrives pre-transposed (D, T); out is written transposed (D, T)
    x_d = nc.dram_tensor("x", [D, T], F32R, kind="ExternalInput")
    lw_d = nc.dram_tensor("layer_w", [L, D, D], F32R, kind="ExternalInput")
    lb_d = nc.dram_tensor("layer_b", [128, L * (D // 128)], F32, kind="ExternalInput")
    hw_d = nc.dram_tensor("halt_w", [128, D // 128], F32R, kind="ExternalInput")
    hb_d = nc.dram_tensor("halt_b", [1, 1], F32, kind="ExternalInput")
    out_d = nc.dram_tensor("out", [D, T], F32, kind="ExternalOutput")
    pond_d = nc.dram_tensor("pond", [R, TR], F32, kind="ExternalOutput")

    with tile.TileContext(nc) as tc:
        with (
            tc.tile_pool(name="big", bufs=1) as big,
            tc.tile_pool(name="wpool", bufs=3) as wpool,
            tc.tile_pool(name="wbcp", bufs=2) as wbcp,
            tc.tile_pool(name="prod", bufs=2) as prod,
            tc.tile_pool(name="rows", bufs=1) as rows,
            tc.tile_pool(name="pmm", bufs=6, space="PSUM") as pmm,
            tc.tile_pool(name="phalt", bufs=2, space="PSUM") as phalt,
        ):
            # persistent SBUF state
            hA = [big.tile([128, TR], F32R, tag=f"hA{i}", name=f"hA{i}") for i in range(ND)]
            hB = [big.tile([128, TR], F32R, tag=f"hB{i}", name=f"hB{i}") for i in range(ND)]
            hC = [big.tile([128, TR], F32R, tag=f"hC{i}", name=f"hC{i}") for i in range(ND)]
            w_res = [big.tile([128, ND, D], F32R, tag=f"wres{s}", name=f"wres{s}")
                     for s in range(2)]
            acc = [big.tile([128, TR], F32, tag=f"acc{i}", name=f"acc{i}") for i in range(ND)]
            b_sb = big.tile([128, L, ND], F32, tag="b_sb", name="b_sb")
            hw_sb = big.tile([128, ND], F32R, tag="hw_sb", name="hw_sb")
            hb_sb = big.tile([1, 1], F32, tag="hb_sb", name="hb_sb")

            cum = rows.tile([1, TR], F32, tag="cum", name="cum")
            rem = rows.tile([1, TR], F32, tag="rem", name="rem")
            pnd = rows.tile([1, TR], F32, tag="pnd", name="pnd")
            p_row = rows.tile([1, TR], F32, tag="p_row", name="p_row")
            w_row = rows.tile([1, TR], F32, tag="w_row", name="w_row")
            t0_row = rows.tile([1, TR], F32, tag="t0_row", name="t0_row")
            t1_row = rows.tile([1, TR], I32, tag="t1_row", name="t1_row")

            bufs3 = [hA, hB, hC]
            for r in range(R):
                # Round r ping-pongs between X[r%3] (x / even h) and
                # X[(r+2)%3]; X[(r+1)%3] is idle all round and receives the
                # NEXT round's x panel, so panel loads never stall the PE.
                pair = [bufs3[r % 3], bufs3[(r + 2) % 3]]
                pp = 0
                nc.vector.memset(cum[:], 0.0)
                nc.vector.memset(rem[:], 1.0)
                nc.vector.memset(pnd[:], 0.0)

                if r == 0:
                    for dt in range(ND):
                        nc.sync.dma_start(
                            bufs3[0][dt][:], x_d[dt * 128:(dt + 1) * 128, 0:TR])
                    for s in range(2):
                        for half in range(2):
                            nc.sync.dma_start(
                                w_res[s][:, half * 4:(half + 1) * 4, :],
                                lw_d[s, half * 512:(half + 1) * 512, :]
                                .rearrange("(dt p) e -> p dt e", p=128))
                if r + 1 < R:
                    nxt = bufs3[(r + 1) % 3]
                    for dt in range(ND):
                        nc.sync.dma_start(
                            nxt[dt][:],
                            x_d[dt * 128:(dt + 1) * 128,
                                (r + 1) * TR:(r + 2) * TR])

                if r == 0:
                    # small params (host-prearranged partition-major), issued
                    # after the x panel so they don't hog the DMA queues
                    nc.sync.dma_start(
                        b_sb[:], lb_d[:].rearrange("p (l eo) -> p l eo", l=L))
                    nc.sync.dma_start(hw_sb[:], hw_d[:])
                    nc.sync.dma_start(hb_sb[:], hb_d[:])

                for s in range(L):
                    cur_src, cur_dst = pair[pp], pair[1 - pp]
                    pp = 1 - pp
                    if s < 2:
                        # W0/W1 are resident
                        wh = [w_res[s][:, 0:4, :], w_res[s][:, 4:8, :]]
                    else:
                        # stream W2/W3 in halves (d-tiles 0-3, 4-7)
                        wh = []
                        for half in range(2):
                            w = wpool.tile(
                                [128, ND // 2, D], F32R, tag="wt", name="wt")
                            nc.sync.dma_start(
                                w[:],
                                lw_d[s, half * 512:(half + 1) * 512, :]
                                .rearrange("(dt p) e -> p dt e", p=128))
                            wh.append(w)

                    # h_new^T = gelu(W^T h^T + b), 128x512 PSUM macro-tiles
                    for eo in range(ND):
                        for c in range(NCH):
                            ps = pmm.tile([128, CH], F32, tag="pmm", name="pmm")
                            for dt in range(ND):
                                nc.tensor.matmul(
                                    ps[:],
                                    wh[dt // 4][:, dt % 4, eo * 128:(eo + 1) * 128],
                                    cur_src[dt][:, c * CH:(c + 1) * CH],
                                    start=(dt == 0), stop=(dt == ND - 1),
                                )
                            nc.scalar.activation(
                                cur_dst[eo][:, c * CH:(c + 1) * CH], ps[:],
                                AF.Gelu_apprx_tanh,
                                bias=b_sb[:, s, eo:eo + 1], scale=1.0)

                    if s < L - 1:
                        # p = sigmoid(h_new @ halt_w + halt_b), (1, TR) row
                        for c in range(NCH):
                            php = phalt.tile([1, CH], F32, tag="phalt", name="phalt")
                            for eo in range(ND):
                                nc.tensor.matmul(
                                    php[:],
                                    hw_sb[:, eo:eo + 1],
                                    cur_dst[eo][:, c * CH:(c + 1) * CH],
                                    start=(eo == 0), stop=(eo == ND - 1),
                                )
                            nc.scalar.activation(
                                p_row[:, c * CH:(c + 1) * CH], php[:],
                                AF.Sigmoid, bias=hb_sb[0:1, 0:1], scale=1.0)

                        # halting state machine on (1, TR) rows
                        nc.vector.tensor_tensor(t0_row[:], cum[:], p_row[:], ALU.add)
                        nc.vector.tensor_scalar(
                            t1_row[:], t0_row[:], THRESHOLD, None, ALU.is_ge)
                        nc.vector.select(w_row[:], t1_row[:], rem[:], p_row[:])
                        nc.vector.tensor_tensor(cum[:], cum[:], w_row[:], ALU.add)
                        nc.vector.tensor_scalar(
                            t0_row[:], cum[:], -1.0, 1.0, ALU.mult, op1=ALU.add)
                        nc.vector.tensor_scalar(
                            rem[:], t0_row[:], 0.0, None, ALU.max)
                        nc.vector.tensor_tensor(pnd[:], pnd[:], w_row[:], ALU.add)
                        wsrc = w_row
                    else:
                        # last step: all tokens forced to halt with weight=rem
                        nc.vector.tensor_tensor(pnd[:], pnd[:], rem[:], ALU.add)
                        wsrc = rem

                    # out^T += w * h_new^T  (w broadcast across partitions)
                    w_bc = wbcp.tile([128, TR], F32, tag="w_bc", name="w_bc")
                    nc.gpsimd.partition_broadcast(w_bc[:], wsrc[:])
                    for eo in range(ND):
                        for c in range(NCH):
                            sl = slice(c * CH, (c + 1) * CH)
                            if s == 0:
                                nc.vector.tensor_mul(
                                    acc[eo][:, sl], w_bc[:, sl],
                                    cur_dst[eo][:, sl].bitcast(F32))
                            else:
                                pr = prod.tile([128, CH], F32, tag="pr", name="pr")
                                nc.vector.tensor_mul(
                                    pr[:], w_bc[:, sl],
                                    cur_dst[eo][:, sl].bitcast(F32))
                                nc.vector.tensor_add(
                                    acc[eo][:, sl], acc[eo][:, sl], pr[:])

                    cur_src, cur_dst = cur_dst, cur_src

                # store out^T panel (host un-transposes)
                for eo in range(ND):
                    nc.sync.dma_start(
                        out_d[eo * 128:(eo + 1) * 128, r * TR:(r + 1) * TR],
                        acc[eo][:])
                nc.sync.dma_start(pond_d[r:r + 1, :], pnd[:])

    nc.compile()
    return nc


def _get_nc():
    global _CACHED
    if _CACHED is None:
        _CACHED = _build()
    return _CACHED


def make_in_maps(x, layer_w, layer_b, halt_w, halt_b):
    """Shard + marshal the full inputs into per-core input maps.

    x is fed transposed (D, T); layer_b / halt_w are pre-arranged
    partition-major so the kernel's small-param DMAs are contiguous.
    """
    x = np.asarray(x, dtype=np.float32)
    layer_w = np.ascontiguousarray(np.asarray(layer_w, dtype=np.float32))
    layer_b = np.asarray(layer_b, dtype=np.float32)
    halt_w = np.asarray(halt_w, dtype=np.float32)
    halt_b = np.ascontiguousarray(
        np.asarray(halt_b, dtype=np.float32)).reshape(1, 1)

    nd = D // 128
    lb = np.ascontiguousarray(
        layer_b.reshape(L, nd, 128).transpose(2, 0, 1).reshape(128, L * nd))
    hw = np.ascontiguousarray(halt_w.reshape(nd, 128).T)

    return [
        {"x": np.ascontiguousarray(x[b].T), "layer_w": layer_w,
         "layer_b": lb, "halt_w": hw, "halt_b": halt_b}
        for b in range(N_CORES)
    ]


def assemble_outputs(results):
    output = np.stack([results[b]["out"].T for b in range(N_CORES)], axis=0)
    ponds = np.stack([results[b]["pond"] for b in range(N_CORES)], axis=0)
    ponder_cost = np.float32(np.mean(ponds.astype(np.float64)))
    return output, ponder_cost


def kernel(x, layer_w, layer_b, halt_w, halt_b):
    from concourse.bass_utils import run_bass_kernel_spmd

    nc = _get_nc()
    in_maps = make_in_maps(x, layer_w, layer_b, halt_w, halt_b)
    res = run_bass_kernel_spmd(nc, in_maps, core_ids=list(range(N_CORES)))
    return assemble_outputs(res.results)


# revision 16
# speedup vs baseline: 1.2133x; 1.2133x over previous
"""ACT (Adaptive Computation Time) transformer layer on 8 TRN2 NeuronCores.

Data-parallel over batch B=8 (one batch row per core). Each core runs the
full 4-step ACT recurrence on its 2048 tokens; the ponder mean is assembled
host-side from per-token ponder rows (no collectives needed).

Per-core layout: h lives TRANSPOSED (D on partitions, tokens on the free
dim) for the whole kernel. The host pre-transposes each core's x shard to
(D, T) and un-transposes the (D, T) output, so the kernel needs no on-chip
transposes at all. Each step's matmul is h_new^T[e,t] = sum_d W[d,e] h^T[d,t]
with W 128x128 blocks as the stationary operand (float32r, full PE rate),
accumulating over d in PSUM; gelu(+bias) is fused into the PSUM->SBUF
eviction on the scalar engine. The halting matvec (D->1) runs on the PE
with M=1 stationary columns, sigmoid fused on its eviction. The per-token
halting state machine runs on (1, T) rows on the DVE; the per-token weight
is broadcast across partitions (gpsimd.partition_broadcast) for the
out^T += w * h^T accumulation. Tokens are processed in 2 panels of 1024 to
fit SBUF; weights stream per (panel, step), double-buffered in halves.
"""

import os
import sys

import numpy as np

if "/opt/trn_rl_repo" not in sys.path:
    sys.path.insert(0, "/opt/trn_rl_repo")

os.environ.setdefault("MYCRO_LOCAL_CACHE", "1")

B, T, D, L = 8, 2048, 1024, 4
THRESHOLD = 0.99
N_CORES = 8

R = 4            # token panels per core
TR = T // R      # tokens per panel
ND = D // 128    # 128-partition tiles along D
CH = 512         # matmul moving free-dim chunk (fp32 PSUM bank)
NCH = TR // CH   # chunks per panel

_CACHED = None


def _build():
    import concourse.bacc as bacc
    import concourse.mybir as mybir
    import concourse.tile as tile

    F32 = mybir.dt.float32
    F32R = mybir.dt.float32r
    I32 = mybir.dt.int32
    AF = mybir.ActivationFunctionType
    ALU = mybir.AluOpType

    nc = bacc.Bacc("TRN2", target_bir_lowering=False, debug=False,
                   num_devices=N_CORES)

    # x arrives pre-transposed (D, T); out is written transposed (D, T)
    x_d = nc.dram_tensor("x", [D, T], F32R, kind="ExternalInput")
    lw_d = nc.dram_tensor("layer_w", [L, D, D], F32R, kind="ExternalInput")
    lb_d = nc.dram_tensor("layer_b", [128, L * (D // 128)], F32, kind="ExternalInput")
    hw_d = nc.dram_tensor("halt_w", [128, D // 128], F32R, kind="ExternalInput")
    hb_d = nc.dram_tensor("halt_b", [1, 1], F32, kind="ExternalInput")
    out_d = nc.dram_tensor("out", [D, T], F32, kind="ExternalOutput")
    pond_d = nc.dram_tensor("pond", [R, TR], F32, kind="ExternalOutput")

    with tile.TileContext(nc) as tc:
        with (
            tc.tile_pool(name="big", bufs=1) as big,
            tc.tile_pool(name="wpool", bufs=3) as wpool,
            tc.tile_pool(name="wbcp", bufs=2) as wbcp,
            tc.tile_pool(name="prod", bufs=2) as prod,
            tc.tile_pool(name="rows", bufs=1) as rows,
            tc.tile_pool(name="pmm", bufs=6, space="PSUM") as pmm,
            tc.tile_pool(name="phalt", bufs=2, space="PSUM") as phalt,
        ):
            # persistent SBUF state
            hA = [big.tile([128, TR], F32R, tag=f"hA{i}", name=f"hA{i}") for i in range(ND)]
            hB = [big.tile([128, TR], F32R, tag=f"hB{i}", name=f"hB{i}") for i in range(ND)]
            hC = [big.tile([128, TR], F32R, tag=f"hC{i}", name=f"hC{i}") for i in range(ND)]
            w_res = [big.tile([128, ND, D], F32R, tag=f"wres{s}", name=f"wres{s}")
                     for s in range(2)]
            acc = [big.tile([128, TR], F32, tag=f"acc{i}", name=f"acc{i}") for i in range(ND)]
            b_sb = big.tile([128, L, ND], F32, tag="b_sb", name="b_sb")
            hw_sb = big.tile([128, ND], F32R, tag="hw_sb", name="hw_sb")
            hb_sb = big.tile([1, 1], F32, tag="hb_sb", name="hb_sb")

            cum = rows.tile([1, TR], F32, tag="cum", name="cum")
            rem = rows.tile([1, TR], F32, tag="rem", name="rem")
            pnd = rows.tile([1, TR], F32, tag="pnd", name="pnd")
            p_row = rows.tile([1, TR], F32, tag="p_row", name="p_row")
            w_row = rows.tile([1, TR], F32, tag="w_row", name="w_row")
            t0_row = rows.tile([1, TR], F32, tag="t0_row", name="t0_row")
            t1_row = rows.tile([1, TR], I32, tag="t1_row", name="t1_row")

            bufs3 = [hA, hB, hC]
            for r in range(R):
                # Round r ping-pongs between X[r%3] (x / even h) and
                # X[(r+2)%3]; X[(r+1)%3] is idle all round and receives the
                # NEXT round's x panel, so panel loads never stall the PE.
                pair = [bufs3[r % 3], bufs3[(r + 2) % 3]]
                pp = 0
                nc.vector.memset(cum[:], 0.0)
                nc.vector.memset(rem[:], 1.0)
                nc.vector.memset(pnd[:], 0.0)

                if r == 0:
                    for dt in range(ND):
                        nc.sync.dma_start(
                            bufs3[0][dt][:], x_d[dt * 128:(dt + 1) * 128, 0:TR])
                    for s in range(2):
                        for half in range(2):
                            nc.sync.dma_start(
                                w_res[s][:, half * 4:(half + 1) * 4, :],
                                lw_d[s, half * 512:(half + 1) * 512, :]
                                .rearrange("(dt p) e -> p dt e", p=128))
                def prefetch_next_x():
                    if r + 1 < R:
                        nxt = bufs3[(r + 1) % 3]
                        for dt in range(ND):
                            nc.sync.dma_start(
                                nxt[dt][:],
                                x_d[dt * 128:(dt + 1) * 128,
                                    (r + 1) * TR:(r + 2) * TR])

                if r > 0:
                    prefetch_next_x()

                if r == 0:
                    # small params (host-prearranged partition-major), issued
                    # after the x panel so they don't hog the DMA queues
                    nc.sync.dma_start(
                        b_sb[:], lb_d[:].rearrange("p (l eo) -> p l eo", l=L))
                    nc.sync.dma_start(hw_sb[:], hw_d[:])
                    nc.sync.dma_start(hb_sb[:], hb_d[:])

                for s in range(L):
                    cur_src, cur_dst = pair[pp], pair[1 - pp]
                    pp = 1 - pp
                    if s < 2:
                        # W0/W1 are resident
                        wh = [w_res[s][:, 0:4, :], w_res[s][:, 4:8, :]]
                    else:
                        # stream W2/W3 in halves (d-tiles 0-3, 4-7)
                        wh = []
                        for half in range(2):
                            w = wpool.tile(
                                [128, ND // 2, D], F32R, tag="wt", name="wt")
                            nc.sync.dma_start(
                                w[:],
                                lw_d[s, half * 512:(half + 1) * 512, :]
                                .rearrange("(dt p) e -> p dt e", p=128))
                            wh.append(w)
                        if r == 0 and s == 2:
                            # round 0's x prefetch yields the DMA queues to W2
                            prefetch_next_x()

                    # h_new^T = gelu(W^T h^T + b), 128x512 PSUM macro-tiles
                    for eo in range(ND):
                        for c in range(NCH):
                            ps = pmm.tile([128, CH], F32, tag="pmm", name="pmm")
                            for dt in range(ND):
                                nc.tensor.matmul(
                                    ps[:],
                                    wh[dt // 4][:, dt % 4, eo * 128:(eo + 1) * 128],
                                    cur_src[dt][:, c * CH:(c + 1) * CH],
                                    start=(dt == 0), stop=(dt == ND - 1),
                                )
                            nc.scalar.activation(
                                cur_dst[eo][:, c * CH:(c + 1) * CH], ps[:],
                                AF.Gelu_apprx_tanh,
                                bias=b_sb[:, s, eo:eo + 1], scale=1.0)

                    if s < L - 1:
                        # p = sigmoid(h_new @ halt_w + halt_b), (1, TR) row
                        for c in range(NCH):
                            php = phalt.tile([1, CH], F32, tag="phalt", name="phalt")
                            for eo in range(ND):
                                nc.tensor.matmul(
                                    php[:],
                                    hw_sb[:, eo:eo + 1],
                                    cur_dst[eo][:, c * CH:(c + 1) * CH],
                                    start=(eo == 0), stop=(eo == ND - 1),
                                )
                            nc.scalar.activation(
                                p_row[:, c * CH:(c + 1) * CH], php[:],
                                AF.Sigmoid, bias=hb_sb[0:1, 0:1], scale=1.0)

                        # halting state machine on (1, TR) rows
                        nc.vector.tensor_tensor(t0_row[:], cum[:], p_row[:], ALU.add)
                        nc.vector.tensor_scalar(
                            t1_row[:], t0_row[:], THRESHOLD, None, ALU.is_ge)
                        nc.vector.select(w_row[:], t1_row[:], rem[:], p_row[:])
                        nc.vector.tensor_tensor(cum[:], cum[:], w_row[:], ALU.add)
                        nc.vector.tensor_scalar(
                            t0_row[:], cum[:], -1.0, 1.0, ALU.mult, op1=ALU.add)
                        nc.vector.tensor_scalar(
                            rem[:], t0_row[:], 0.0, None, ALU.max)
                        nc.vector.tensor_tensor(pnd[:], pnd[:], w_row[:], ALU.add)
                        wsrc = w_row
                    else:
                        # last step: all tokens forced to halt with weight=rem
                        nc.vector.tensor_tensor(pnd[:], pnd[:], rem[:], ALU.add)
                        wsrc = rem

                    # out^T += w * h_new^T  (w broadcast across partitions)
                    w_bc = wbcp.tile([128, TR], F32, tag="w_bc", name="w_bc")
                    nc.gpsimd.partition_broadcast(w_bc[:], wsrc[:])
                    for eo in range(ND):
                        for c in range(NCH):
                            sl = slice(c * CH, (c + 1) * CH)
                            if s == 0:
                                nc.vector.tensor_mul(
                                    acc[eo][:, sl], w_bc[:, sl],
                                    cur_dst[eo][:, sl].bitcast(F32))
                            else:
                                pr = prod.tile([128, CH], F32, tag="pr", name="pr")
                                nc.vector.tensor_mul(
                                    pr[:], w_bc[:, sl],
                                    cur_dst[eo][:, sl].bitcast(F32))
                                nc.vector.tensor_add(
                                    acc[eo][:, sl], acc[eo][:, sl], pr[:])

                    cur_src, cur_dst = cur_dst, cur_src

                # store out^T panel (host un-transposes)
                for eo in range(ND):
                    nc.sync.dma_start(
                        out_d[eo * 128:(eo + 1) * 128, r * TR:(r + 1) * TR],
                        acc[eo][:])
                nc.sync.dma_start(pond_d[r:r + 1, :], pnd[:])

    nc.compile()
    return nc


def _get_nc():
    global _CACHED
    if _CACHED is None:
        _CACHED = _build()
    return _CACHED


def make_in_maps(x, layer_w, layer_b, halt_w, halt_b):
    """Shard + marshal the full inputs into per-core input maps.

    x is fed transposed (D, T); layer_b / halt_w are pre-arranged
    partition-major so the kernel's small-param DMAs are contiguous.
    """
    x = np.asarray(x, dtype=np.float32)
    layer_w = np.ascontiguousarray(np.asarray(layer_w, dtype=np.float32))
    layer_b = np.asarray(layer_b, dtype=np.float32)
    halt_w = np.asarray(halt_w, dtype=np.float32)
    halt_b = np.ascontiguousarray(
        np.asarray(halt_b, dtype=np.float32)).reshape(1, 1)

    nd = D // 128
    lb = np.ascontiguousarray(
        layer_b.reshape(L, nd, 128).transpose(2, 0, 1).reshape(128, L * nd))
    hw = np.ascontiguousarray(halt_w.reshape(nd, 128).T)

    return [
        {"x": np.ascontiguousarray(x[b].T), "layer_w": layer_w,
         "layer_b": lb, "halt_w": hw, "halt_b": halt_b}
        for b in range(N_CORES)
    ]


def assemble_outputs(results):
    output = np.stack([results[b]["out"].T for b in range(N_CORES)], axis=0)
    ponds = np.stack([results[b]["pond"] for b in range(N_CORES)], axis=0)
    ponder_cost = np.float32(np.mean(ponds.astype(np.float64)))
    return output, ponder_cost


def kernel(x, layer_w, layer_b, halt_w, halt_b):
    from concourse.bass_utils import run_bass_kernel_spmd

    nc = _get_nc()
    in_maps = make_in_maps(x, layer_w, layer_b, halt_w, halt_b)
    res = run_bass_kernel_spmd(nc, in_maps, core_ids=list(range(N_CORES)))
    return assemble_outputs(res.results)
